# revision 9
# baseline (speedup 1.0000x reference)
"""Trainium2 Bass kernel for nn_AttentionSubsample (8-core SPMD).

Sharding: batch N=2 x 4 head-groups (3 heads each) -> 8 cores, no
collectives.  Each core computes q/k/v projections for its head group
(K/V on the stride-2 subsampled positions only), per-head attention with
softmax folded as exp -> denominator via an appended ones-column in V ->
divide, and its partial output projection in transposed layout.  The
host sums the 4 per-batch partials and adds the bias.

Layout notes:
 - The spatial stride-2 subsample of K/V equals taking even rows of the
   flattened [3136, 768] batch (196 is even), i.e. even columns of x^T.
 - All device matmuls run in bf16 (fp32 PSUM accumulation).
"""

import sys

for _p in ("/opt/trn_rl_repo",):
    if _p not in sys.path:
        sys.path.insert(0, _p)

import numpy as np
import ml_dtypes

import concourse.bass as bass  # noqa: F401  (registers engines)
import concourse.tile as tile
from concourse import bacc, mybir
from concourse.bass_utils import run_bass_kernel_spmd

BFNP = ml_dtypes.bfloat16
F32 = mybir.dt.float32
F32R = mybir.dt.float32r
BF16 = mybir.dt.bfloat16
AF = mybir.ActivationFunctionType

N, T, S, D = 2, 16, 196, 768
H, HD = 12, 64
Q = T * S              # 3136 query positions per batch
KP = T * (S // 2)      # 1568 subsampled key positions
HPG = 3                # heads per group (12 heads / 4 groups)
GD = HPG * HD          # 192 channels per head group
SC = (D // H) ** -0.5  # 0.125 attention scale
CH = 448               # q-chunk size (3136 = 7 * 448)
NCH = Q // CH          # 7
NKT = 13               # k tiles: 12 * 128 + 32
KTL = 32               # last k-tile height
NDK = D // 128         # 6 contraction tiles for the projections
N_CORES = 8

# exp groups over k-tiles: PSUM scores tile holds 3 banks (512-aligned)
EXP_GROUPS = [(0, 1, 2), (3, 4, 5), (6, 7, 8), (9, 10, 11), (12,)]

TRACE = False          # test.py flips this for profiled runs
LAST_RESULTS = {}      # exec_time_ns etc. stashed here on traced runs

_CACHE = {}


def _ksize(kt):
    return 128 if kt < NKT - 1 else KTL


def _head_pos(h):
    """(block, partition base) of head h inside the 2-block qT/kT tiles."""
    return (0, 0) if h == 0 else ((0, 64) if h == 1 else (1, 0))


def _build_nc():
    nc = bacc.Bacc(
        "TRN2", target_bir_lowering=False, debug=False, num_devices=N_CORES
    )
    xT = nc.dram_tensor("xT", [D, Q], BF16, kind="ExternalInput").ap()
    wq = nc.dram_tensor("wq", [D, GD], BF16, kind="ExternalInput").ap()
    wk = nc.dram_tensor("wk", [D, GD], BF16, kind="ExternalInput").ap()
    wv = nc.dram_tensor("wv", [D, GD], BF16, kind="ExternalInput").ap()
    wp = nc.dram_tensor("wp", [GD, D], BF16, kind="ExternalInput").ap()
    out = nc.dram_tensor("out", [D, Q], F32, kind="ExternalOutput").ap()

    with tile.TileContext(nc) as tc:
        _body(tc, xT, wq, wk, wv, wp, out)
    nc.compile()
    return nc


def _body(tc, xT, wq, wk, wv, wp, out):
    nc = tc.nc
    with (
        tc.tile_pool(name="persist", bufs=1) as P,
        tc.tile_pool(name="es", bufs=3) as ES,
        tc.tile_pool(name="inv", bufs=2) as INV,
        tc.tile_pool(name="ot", bufs=3) as OT,
        tc.tile_pool(name="scps", bufs=2, space="PSUM") as SCPS,
        tc.tile_pool(name="numps", bufs=1, space="PSUM") as NUMPS,
        tc.tile_pool(name="miscps", bufs=1, space="PSUM") as MISCPS,
    ):
        # ---- persistent SBUF tensors -------------------------------------
        xt = P.tile([128, NDK * Q], BF16, tag="xt")
        nc.sync.dma_start(
            xt[:].rearrange("p (a q) -> p a q", a=NDK),
            xT.rearrange("(a p) q -> p a q", p=128),
        )
        wq_sb = P.tile([128, NDK * GD], BF16, tag="wq")
        nc.sync.dma_start(
            wq_sb[:].rearrange("p (a g) -> p a g", a=NDK),
            wq.rearrange("(a p) g -> p a g", p=128),
        )
        wk_sb = P.tile([128, NDK * GD], BF16, tag="wk")
        nc.sync.dma_start(
            wk_sb[:].rearrange("p (a g) -> p a g", a=NDK),
            wk.rearrange("(a p) g -> p a g", p=128),
        )
        wv_sb = P.tile([128, NDK * GD], BF16, tag="wv")
        nc.sync.dma_start(
            wv_sb[:].rearrange("p (a g) -> p a g", a=NDK),
            wv.rearrange("(a p) g -> p a g", p=128),
        )
        # wp: h0 rows at partitions 0:64 and h1 at 64:128 of block 0 (so the
        # h0+h1 pair contracts as one K=128 matmul); h2 in block 1.
        wp_sb = P.tile([128, 2 * D], BF16, tag="wp")
        nc.sync.dma_start(wp_sb[0:HD, 0:D], wp[0:HD, :])
        nc.sync.dma_start(wp_sb[HD:128, 0:D], wp[HD : 2 * HD, :])
        nc.sync.dma_start(wp_sb[0:HD, D : 2 * D], wp[2 * HD : 3 * HD, :])
        ones_f = P.tile([128, HD], F32, tag="ones_f")
        nc.vector.memset(ones_f[:], 1.0)
        ones = P.tile([128, HD], F32R, tag="ones")
        nc.vector.tensor_copy(ones[:], ones_f[:])

        qT = P.tile([128, 2 * Q], BF16, tag="qT")       # q^T: rows=[h0|h1], [h2]
        kT = P.tile([128, 2 * KP], BF16, tag="kT")      # k^T subsampled
        vv = P.tile([128, HPG * NKT * 65], BF16, tag="v")  # v + ones col, [k, 65]/tile
        # attn out: block 0 rows 0:64 = h0, rows 64:128 = h1; block 1 = h2
        attn = P.tile([128, 2 * Q], BF16, tag="attn")

        # ones columns of the v tiles
        nc.vector.memset(vv[:, 64 : HPG * NKT * 65 : 65], 1.0)

        # ---- A: projections ---------------------------------------------
        # A1: q^T [GD, Q] = wq^T @ x^T   (m-blocks of 128/64 rows)
        for m, msz in ((0, 128), (1, 64)):
            for c in range(NCH):
                ps = SCPS.tile([128, 448], F32, tag="sc")
                for kt in range(NDK):
                    nc.tensor.matmul(
                        ps[0:msz, 0:CH],
                        wq_sb[:, kt * GD + m * 128 : kt * GD + m * 128 + msz],
                        xt[:, kt * Q + c * CH : kt * Q + (c + 1) * CH],
                        start=(kt == 0),
                        stop=(kt == NDK - 1),
                    )
                nc.vector.tensor_copy(
                    qT[0:msz, m * Q + c * CH : m * Q + (c + 1) * CH],
                    ps[0:msz, 0:CH],
                )
        # A2: k^T [GD, KP] = wk^T @ x_sub^T  (even columns of x^T)
        kchunks = [(0, 448), (448, 448), (896, 448), (1344, 224)]
        for m, msz in ((0, 128), (1, 64)):
            for c0, csz in kchunks:
                ps = SCPS.tile([128, 448], F32, tag="sc")
                for kt in range(NDK):
                    base = kt * Q
                    nc.tensor.matmul(
                        ps[0:msz, 0:csz],
                        wk_sb[:, kt * GD + m * 128 : kt * GD + m * 128 + msz],
                        xt[:, base + 2 * c0 : base + 2 * (c0 + csz) : 2],
                        start=(kt == 0),
                        stop=(kt == NDK - 1),
                    )
                nc.vector.tensor_copy(
                    kT[0:msz, m * KP + c0 : m * KP + c0 + csz],
                    ps[0:msz, 0:csz],
                )
        # A3: v [KP, GD] = x_sub @ wv, natural layout, per (head, ktile) slots
        for kt_m in range(NKT):
            msz = _ksize(kt_m)
            ps = SCPS.tile([128, 448], F32, tag="sc")
            for kt in range(NDK):
                base = kt * Q + 2 * (kt_m * 128)
                nc.tensor.matmul(
                    ps[0:msz, 0:GD],
                    xt[:, base : base + 2 * msz : 2],
                    wv_sb[:, kt * GD : (kt + 1) * GD],
                    start=(kt == 0),
                    stop=(kt == NDK - 1),
                )
            for h in range(HPG):
                slot = (h * NKT + kt_m) * 65
                nc.vector.tensor_copy(
                    vv[0:msz, slot : slot + 64],
                    ps[0:msz, h * HD : (h + 1) * HD],
                )

        # ---- B/C: attention per q-chunk ----------------------------------
        def scores_and_exp(h, c, es):
            blk, pb = _head_pos(h)
            q_rhs = qT[pb : pb + HD, blk * Q + c * CH : blk * Q + (c + 1) * CH]
            for grp in EXP_GROUPS:
                scp = SCPS.tile([128, 1536], F32, tag="sc")
                pmax = _ksize(grp[-1])
                for j, kt in enumerate(grp):
                    ksz = _ksize(kt)
                    nc.tensor.matmul(
                        scp[0:ksz, j * 512 : j * 512 + CH],
                        kT[pb : pb + HD, blk * KP + kt * 128 : blk * KP + kt * 128 + ksz],
                        q_rhs,
                        start=True,
                        stop=True,
                    )
                ng = len(grp)
                src = scp[0:pmax, 0 : ng * 512].rearrange(
                    "p (a b) -> p a b", b=512
                )[:, :, 0:CH] if ng > 1 else scp[0:pmax, 0:CH]
                dst = es[
                    0:pmax, grp[0] * CH : (grp[-1] + 1) * CH
                ].rearrange("p (a b) -> p a b", b=CH) if ng > 1 else es[
                    0:pmax, grp[0] * CH : grp[0] * CH + CH
                ]
                nc.scalar.activation(dst, src, AF.Exp, scale=SC)

        def scores_and_exp_pair(c, es0, es1):
            # heads 0 and 1 interleaved: their operands live at partition
            # bases 0 and 64, so consecutive matmuls land on different PE
            # row groups and overlap in the array.
            q0 = qT[0:HD, c * CH : (c + 1) * CH]
            q1 = qT[HD:128, c * CH : (c + 1) * CH]
            for grp in EXP_GROUPS:
                scp0 = SCPS.tile([128, 1536], F32, tag="sc")
                scp1 = SCPS.tile([128, 1536], F32, tag="sc")
                pmax = _ksize(grp[-1])
                for j, kt in enumerate(grp):
                    ksz = _ksize(kt)
                    nc.tensor.matmul(
                        scp0[0:ksz, j * 512 : j * 512 + CH],
                        kT[0:HD, kt * 128 : kt * 128 + ksz],
                        q0,
                        start=True,
                        stop=True,
                    )
                    nc.tensor.matmul(
                        scp1[0:ksz, j * 512 : j * 512 + CH],
                        kT[HD:128, kt * 128 : kt * 128 + ksz],
                        q1,
                        start=True,
                        stop=True,
                    )
                ng = len(grp)
                for scp, es in ((scp0, es0), (scp1, es1)):
                    src = scp[0:pmax, 0 : ng * 512].rearrange(
                        "p (a b) -> p a b", b=512
                    )[:, :, 0:CH] if ng > 1 else scp[0:pmax, 0:CH]
                    dst = es[
                        0:pmax, grp[0] * CH : (grp[-1] + 1) * CH
                    ].rearrange("p (a b) -> p a b", b=CH) if ng > 1 else es[
                        0:pmax, grp[0] * CH : grp[0] * CH + CH
                    ]
                    nc.scalar.activation(dst, src, AF.Exp, scale=SC)

        def pv_and_div(h, c, es):
            # out partition rows: h0 -> attn[0:64] blk0, h1 -> attn[64:128]
            # blk0, h2 -> attn[0:64] blk1
            num = NUMPS.tile([128, 512], F32, tag="num")
            for kt in range(NKT):
                ksz = _ksize(kt)
                slot = (h * NKT + kt) * 65
                nc.tensor.matmul(
                    num[0:65, 0:CH],
                    vv[0:ksz, slot : slot + 65],
                    es[0:ksz, kt * CH : (kt + 1) * CH],
                    start=(kt == 0),
                    stop=(kt == NKT - 1),
                )
            den = INV.tile([128, CH], F32, tag="den")
            nc.vector.tensor_copy(den[0:1, :], num[64:65, 0:CH])
            inv = INV.tile([128, CH], F32, tag="inv")
            nc.vector.reciprocal_approx_fast(inv[0:1, :], den[0:1, :])
            invr = INV.tile([128, CH], F32R, tag="invr")
            nc.vector.tensor_copy(invr[0:1, :], inv[0:1, :])
            rep = MISCPS.tile([128, 512], F32, tag="rep")
            nc.tensor.matmul(
                rep[0:64, 0:CH], ones[0:1, 0:64], invr[0:1, 0:CH],
                start=True, stop=True,
            )
            nsb = INV.tile([128, CH], F32, tag="nsb")
            nc.vector.tensor_copy(nsb[0:64, :], num[0:64, 0:CH])
            if h == 0:
                dst = attn[0:64, c * CH : (c + 1) * CH]
            elif h == 1:
                dst = attn[64:128, c * CH : (c + 1) * CH]
            else:
                dst = attn[0:64, Q + c * CH : Q + (c + 1) * CH]
            nc.vector.tensor_tensor(
                dst, nsb[0:64, :], rep[0:64, 0:CH], op=mybir.AluOpType.mult
            )

        for c in range(NCH):
            es0 = ES.tile([128, NKT * CH], BF16, tag="es")
            es1 = ES.tile([128, NKT * CH], BF16, tag="es")
            scores_and_exp_pair(c, es0, es1)
            es2 = ES.tile([128, NKT * CH], BF16, tag="es")
            scores_and_exp(2, c, es2)
            pv_and_div(0, c, es0)
            pv_and_div(1, c, es1)
            pv_and_div(2, c, es2)

        # ---- D: output projection (transposed partial) -------------------
        for m in range(NDK):
            for c in range(NCH):
                pp = SCPS.tile([128, 448], F32, tag="sc")
                nc.tensor.matmul(
                    pp[0:128, 0:CH],
                    wp_sb[0:128, m * 128 : (m + 1) * 128],
                    attn[0:128, c * CH : (c + 1) * CH],
                    start=True,
                    stop=False,
                )
                nc.tensor.matmul(
                    pp[0:128, 0:CH],
                    wp_sb[0:HD, D + m * 128 : D + (m + 1) * 128],
                    attn[0:HD, Q + c * CH : Q + (c + 1) * CH],
                    start=False,
                    stop=True,
                )
                ot = OT.tile([128, CH], F32, tag="ot")
                nc.vector.tensor_copy(ot[:], pp[0:128, 0:CH])
                nc.sync.dma_start(
                    out[m * 128 : (m + 1) * 128, c * CH : (c + 1) * CH], ot[:]
                )


def _get_nc():
    if "nc" not in _CACHE:
        _CACHE["nc"] = _build_nc()
    return _CACHE["nc"]


def kernel(x, W_qkv, W_proj, b_proj):
    nc = _get_nc()
    xTs = [
        np.ascontiguousarray(
            x[n].reshape(Q, D).astype(BFNP).T
        )
        for n in range(N)
    ]
    wqs, wks, wvs, wps = [], [], [], []
    for g in range(4):
        c0 = g * GD
        wqs.append(np.ascontiguousarray(W_qkv[:, c0 : c0 + GD].astype(BFNP)))
        wks.append(np.ascontiguousarray(W_qkv[:, D + c0 : D + c0 + GD].astype(BFNP)))
        wvs.append(
            np.ascontiguousarray(W_qkv[:, 2 * D + c0 : 2 * D + c0 + GD].astype(BFNP))
        )
        wps.append(np.ascontiguousarray(W_proj[c0 : c0 + GD, :].astype(BFNP)))
    in_maps = [
        {"xT": xTs[c // 4], "wq": wqs[c % 4], "wk": wks[c % 4],
         "wv": wvs[c % 4], "wp": wps[c % 4]}
        for c in range(N_CORES)
    ]
    res = run_bass_kernel_spmd(nc, in_maps, list(range(N_CORES)), trace=TRACE)
    if TRACE:
        LAST_RESULTS["exec_time_ns"] = res.exec_time_ns
        LAST_RESULTS["mean_exec_time_ns"] = res.mean_exec_time_ns
    out = np.empty((N, T, S, D), np.float32)
    for n in range(N):
        acc = res.results[4 * n]["out"]
        for g in range(1, 4):
            acc = acc + res.results[4 * n + g]["out"]
        out[n] = (acc.T + b_proj).reshape(T, S, D)
    return out


# revision 10
# speedup vs baseline: 1.1572x; 1.1572x over previous
"""Trainium2 Bass kernel for nn_AttentionSubsample (8-core SPMD).

Sharding: batch N=2 x 4 head-groups (3 heads each) -> 8 cores, no
collectives.  Each core computes q/k/v projections for its head group
(K/V on the stride-2 subsampled positions only), per-head attention with
softmax folded as exp -> denominator via an appended ones-column in V ->
divide, and its partial output projection in transposed layout.  The
host sums the 4 per-batch partials and adds the bias.

Layout notes:
 - The spatial stride-2 subsample of K/V equals taking even rows of the
   flattened [3136, 768] batch (196 is even), i.e. even columns of x^T.
 - All device matmuls run in bf16 (fp32 PSUM accumulation).
"""

import sys

for _p in ("/opt/trn_rl_repo",):
    if _p not in sys.path:
        sys.path.insert(0, _p)

import numpy as np
import ml_dtypes

import concourse.bass as bass  # noqa: F401  (registers engines)
import concourse.tile as tile
from concourse import bacc, mybir
from concourse.bass_utils import run_bass_kernel_spmd

BFNP = ml_dtypes.bfloat16
F32 = mybir.dt.float32
F32R = mybir.dt.float32r
BF16 = mybir.dt.bfloat16
AF = mybir.ActivationFunctionType

N, T, S, D = 2, 16, 196, 768
H, HD = 12, 64
Q = T * S              # 3136 query positions per batch
KP = T * (S // 2)      # 1568 subsampled key positions
HPG = 3                # heads per group (12 heads / 4 groups)
GD = HPG * HD          # 192 channels per head group
SC = (D // H) ** -0.5  # 0.125 attention scale
CH = 448               # q-chunk size (3136 = 7 * 448)
NCH = Q // CH          # 7
NKT = 13               # k tiles: 12 * 128 + 32
KTL = 32               # last k-tile height
NDK = D // 128         # 6 contraction tiles for the projections
N_CORES = 8

# exp groups over k-tiles: PSUM scores tile holds 3 banks (512-aligned)
EXP_GROUPS = [(0, 1, 2), (3, 4, 5), (6, 7, 8), (9, 10, 11), (12,)]

TRACE = False          # test.py flips this for profiled runs
LAST_RESULTS = {}      # exec_time_ns etc. stashed here on traced runs

_CACHE = {}


def _ksize(kt):
    return 128 if kt < NKT - 1 else KTL


def _head_pos(h):
    """(block, partition base) of head h inside the 2-block qT/kT tiles."""
    return (0, 0) if h == 0 else ((0, 64) if h == 1 else (1, 0))


def _build_nc():
    nc = bacc.Bacc(
        "TRN2", target_bir_lowering=False, debug=False, num_devices=N_CORES
    )
    xT = nc.dram_tensor("xT", [D, Q], BF16, kind="ExternalInput").ap()
    wq = nc.dram_tensor("wq", [D, GD], BF16, kind="ExternalInput").ap()
    wk = nc.dram_tensor("wk", [D, GD], BF16, kind="ExternalInput").ap()
    wv = nc.dram_tensor("wv", [D, GD], BF16, kind="ExternalInput").ap()
    wp = nc.dram_tensor("wp", [GD, D], BF16, kind="ExternalInput").ap()
    out = nc.dram_tensor("out", [D, Q], F32, kind="ExternalOutput").ap()

    with tile.TileContext(nc) as tc:
        _body(tc, xT, wq, wk, wv, wp, out)
    nc.compile()
    return nc


def _body(tc, xT, wq, wk, wv, wp, out):
    nc = tc.nc
    with (
        tc.tile_pool(name="persist", bufs=1) as P,
        tc.tile_pool(name="es", bufs=6) as ES,
        tc.tile_pool(name="inv", bufs=2) as INV,
        tc.tile_pool(name="ot", bufs=3) as OT,
        tc.tile_pool(name="scps", bufs=2, space="PSUM") as SCPS,
        tc.tile_pool(name="numps", bufs=1, space="PSUM") as NUMPS,
        tc.tile_pool(name="miscps", bufs=1, space="PSUM") as MISCPS,
    ):
        # ---- persistent SBUF tensors -------------------------------------
        xt = P.tile([128, NDK * Q], BF16, tag="xt")
        nc.sync.dma_start(
            xt[:].rearrange("p (a q) -> p a q", a=NDK),
            xT.rearrange("(a p) q -> p a q", p=128),
        )
        wq_sb = P.tile([128, NDK * GD], BF16, tag="wq")
        nc.sync.dma_start(
            wq_sb[:].rearrange("p (a g) -> p a g", a=NDK),
            wq.rearrange("(a p) g -> p a g", p=128),
        )
        wk_sb = P.tile([128, NDK * GD], BF16, tag="wk")
        nc.sync.dma_start(
            wk_sb[:].rearrange("p (a g) -> p a g", a=NDK),
            wk.rearrange("(a p) g -> p a g", p=128),
        )
        wv_sb = P.tile([128, NDK * GD], BF16, tag="wv")
        nc.sync.dma_start(
            wv_sb[:].rearrange("p (a g) -> p a g", a=NDK),
            wv.rearrange("(a p) g -> p a g", p=128),
        )
        # wp: h0 rows at partitions 0:64 and h1 at 64:128 of block 0 (so the
        # h0+h1 pair contracts as one K=128 matmul); h2 in block 1.
        wp_sb = P.tile([128, 2 * D], BF16, tag="wp")
        nc.sync.dma_start(wp_sb[0:HD, 0:D], wp[0:HD, :])
        nc.sync.dma_start(wp_sb[HD:128, 0:D], wp[HD : 2 * HD, :])
        nc.sync.dma_start(wp_sb[0:HD, D : 2 * D], wp[2 * HD : 3 * HD, :])
        ones_f = P.tile([128, HD], F32, tag="ones_f")
        nc.vector.memset(ones_f[:], 1.0)
        ones = P.tile([128, HD], F32R, tag="ones")
        nc.vector.tensor_copy(ones[:], ones_f[:])

        qT = P.tile([128, 2 * Q], BF16, tag="qT")       # q^T: rows=[h0|h1], [h2]
        kT = P.tile([128, 2 * KP], BF16, tag="kT")      # k^T subsampled
        vv = P.tile([128, HPG * NKT * 65], BF16, tag="v")  # v + ones col, [k, 65]/tile
        # attn out: block 0 rows 0:64 = h0, rows 64:128 = h1; block 1 = h2
        attn = P.tile([128, 2 * Q], BF16, tag="attn")

        # ones columns of the v tiles
        nc.vector.memset(vv[:, 64 : HPG * NKT * 65 : 65], 1.0)

        # ---- A: projections ---------------------------------------------
        # A1: q^T [GD, Q] = wq^T @ x^T   (m-blocks of 128/64 rows)
        for m, msz in ((0, 128), (1, 64)):
            for c in range(NCH):
                ps = SCPS.tile([128, 448], F32, tag="sc")
                for kt in range(NDK):
                    nc.tensor.matmul(
                        ps[0:msz, 0:CH],
                        wq_sb[:, kt * GD + m * 128 : kt * GD + m * 128 + msz],
                        xt[:, kt * Q + c * CH : kt * Q + (c + 1) * CH],
                        start=(kt == 0),
                        stop=(kt == NDK - 1),
                    )
                nc.vector.tensor_copy(
                    qT[0:msz, m * Q + c * CH : m * Q + (c + 1) * CH],
                    ps[0:msz, 0:CH],
                )
        # A2: k^T [GD, KP] = wk^T @ x_sub^T  (even columns of x^T)
        kchunks = [(0, 448), (448, 448), (896, 448), (1344, 224)]
        for m, msz in ((0, 128), (1, 64)):
            for c0, csz in kchunks:
                ps = SCPS.tile([128, 448], F32, tag="sc")
                for kt in range(NDK):
                    base = kt * Q
                    nc.tensor.matmul(
                        ps[0:msz, 0:csz],
                        wk_sb[:, kt * GD + m * 128 : kt * GD + m * 128 + msz],
                        xt[:, base + 2 * c0 : base + 2 * (c0 + csz) : 2],
                        start=(kt == 0),
                        stop=(kt == NDK - 1),
                    )
                nc.vector.tensor_copy(
                    kT[0:msz, m * KP + c0 : m * KP + c0 + csz],
                    ps[0:msz, 0:csz],
                )
        # A3: v [KP, GD] = x_sub @ wv, natural layout, per (head, ktile) slots
        for kt_m in range(NKT):
            msz = _ksize(kt_m)
            ps = SCPS.tile([128, 448], F32, tag="sc")
            for kt in range(NDK):
                base = kt * Q + 2 * (kt_m * 128)
                nc.tensor.matmul(
                    ps[0:msz, 0:GD],
                    xt[:, base : base + 2 * msz : 2],
                    wv_sb[:, kt * GD : (kt + 1) * GD],
                    start=(kt == 0),
                    stop=(kt == NDK - 1),
                )
            for h in range(HPG):
                slot = (h * NKT + kt_m) * 65
                nc.vector.tensor_copy(
                    vv[0:msz, slot : slot + 64],
                    ps[0:msz, h * HD : (h + 1) * HD],
                )

        # ---- B/C: attention per q-chunk ----------------------------------
        def scores_and_exp(h, c, es):
            blk, pb = _head_pos(h)
            q_rhs = qT[pb : pb + HD, blk * Q + c * CH : blk * Q + (c + 1) * CH]
            for grp in EXP_GROUPS:
                scp = SCPS.tile([128, 1536], F32, tag="sc")
                pmax = _ksize(grp[-1])
                for j, kt in enumerate(grp):
                    ksz = _ksize(kt)
                    nc.tensor.matmul(
                        scp[0:ksz, j * 512 : j * 512 + CH],
                        kT[pb : pb + HD, blk * KP + kt * 128 : blk * KP + kt * 128 + ksz],
                        q_rhs,
                        start=True,
                        stop=True,
                    )
                ng = len(grp)
                src = scp[0:pmax, 0 : ng * 512].rearrange(
                    "p (a b) -> p a b", b=512
                )[:, :, 0:CH] if ng > 1 else scp[0:pmax, 0:CH]
                dst = es[
                    0:pmax, grp[0] * CH : (grp[-1] + 1) * CH
                ].rearrange("p (a b) -> p a b", b=CH) if ng > 1 else es[
                    0:pmax, grp[0] * CH : grp[0] * CH + CH
                ]
                nc.scalar.activation(dst, src, AF.Exp, scale=SC)

        def scores_and_exp_pair(c, es0, es1):
            # heads 0 and 1 interleaved: their operands live at partition
            # bases 0 and 64, so consecutive matmuls land on different PE
            # row groups and overlap in the array.
            q0 = qT[0:HD, c * CH : (c + 1) * CH]
            q1 = qT[HD:128, c * CH : (c + 1) * CH]
            for grp in EXP_GROUPS:
                scp0 = SCPS.tile([128, 1536], F32, tag="sc")
                scp1 = SCPS.tile([128, 1536], F32, tag="sc")
                pmax = _ksize(grp[-1])
                for j, kt in enumerate(grp):
                    ksz = _ksize(kt)
                    nc.tensor.matmul(
                        scp0[0:ksz, j * 512 : j * 512 + CH],
                        kT[0:HD, kt * 128 : kt * 128 + ksz],
                        q0,
                        start=True,
                        stop=True,
                    )
                    nc.tensor.matmul(
                        scp1[0:ksz, j * 512 : j * 512 + CH],
                        kT[HD:128, kt * 128 : kt * 128 + ksz],
                        q1,
                        start=True,
                        stop=True,
                    )
                ng = len(grp)
                for scp, es in ((scp0, es0), (scp1, es1)):
                    src = scp[0:pmax, 0 : ng * 512].rearrange(
                        "p (a b) -> p a b", b=512
                    )[:, :, 0:CH] if ng > 1 else scp[0:pmax, 0:CH]
                    dst = es[
                        0:pmax, grp[0] * CH : (grp[-1] + 1) * CH
                    ].rearrange("p (a b) -> p a b", b=CH) if ng > 1 else es[
                        0:pmax, grp[0] * CH : grp[0] * CH + CH
                    ]
                    nc.scalar.activation(dst, src, AF.Exp, scale=SC)

        def pv_and_div(h, c, es):
            # out partition rows: h0 -> attn[0:64] blk0, h1 -> attn[64:128]
            # blk0, h2 -> attn[0:64] blk1
            num = NUMPS.tile([128, 512], F32, tag="num")
            for kt in range(NKT):
                ksz = _ksize(kt)
                slot = (h * NKT + kt) * 65
                nc.tensor.matmul(
                    num[0:65, 0:CH],
                    vv[0:ksz, slot : slot + 65],
                    es[0:ksz, kt * CH : (kt + 1) * CH],
                    start=(kt == 0),
                    stop=(kt == NKT - 1),
                )
            den = INV.tile([128, CH], F32, tag="den")
            nc.vector.tensor_copy(den[0:1, :], num[64:65, 0:CH])
            inv = INV.tile([128, CH], F32, tag="inv")
            nc.vector.reciprocal_approx_fast(inv[0:1, :], den[0:1, :])
            invr = INV.tile([128, CH], F32R, tag="invr")
            nc.vector.tensor_copy(invr[0:1, :], inv[0:1, :])
            rep = MISCPS.tile([128, 512], F32, tag="rep")
            nc.tensor.matmul(
                rep[0:64, 0:CH], ones[0:1, 0:64], invr[0:1, 0:CH],
                start=True, stop=True,
            )
            nsb = INV.tile([128, CH], F32, tag="nsb")
            nc.vector.tensor_copy(nsb[0:64, :], num[0:64, 0:CH])
            if h == 0:
                dst = attn[0:64, c * CH : (c + 1) * CH]
            elif h == 1:
                dst = attn[64:128, c * CH : (c + 1) * CH]
            else:
                dst = attn[0:64, Q + c * CH : Q + (c + 1) * CH]
            nc.vector.tensor_tensor(
                dst, nsb[0:64, :], rep[0:64, 0:CH], op=mybir.AluOpType.mult
            )

        # software pipeline: PE runs chunk c-1's PV/div while ACT
        # exponentiates chunk c's scores (keeps the PE stream stall-free
        # so the HAM clock gate stays at full rate)
        pv_queue = []
        for c in range(NCH):
            es0 = ES.tile([128, NKT * CH], BF16, tag="es")
            es1 = ES.tile([128, NKT * CH], BF16, tag="es")
            scores_and_exp_pair(c, es0, es1)
            es2 = ES.tile([128, NKT * CH], BF16, tag="es")
            scores_and_exp(2, c, es2)
            for h, pc, pes in pv_queue:
                pv_and_div(h, pc, pes)
            pv_queue = [(0, c, es0), (1, c, es1), (2, c, es2)]
        for h, pc, pes in pv_queue:
            pv_and_div(h, pc, pes)

        # ---- D: output projection (transposed partial) -------------------
        for m in range(NDK):
            for c in range(NCH):
                pp = SCPS.tile([128, 448], F32, tag="sc")
                nc.tensor.matmul(
                    pp[0:128, 0:CH],
                    wp_sb[0:128, m * 128 : (m + 1) * 128],
                    attn[0:128, c * CH : (c + 1) * CH],
                    start=True,
                    stop=False,
                )
                nc.tensor.matmul(
                    pp[0:128, 0:CH],
                    wp_sb[0:HD, D + m * 128 : D + (m + 1) * 128],
                    attn[0:HD, Q + c * CH : Q + (c + 1) * CH],
                    start=False,
                    stop=True,
                )
                ot = OT.tile([128, CH], F32, tag="ot")
                nc.vector.tensor_copy(ot[:], pp[0:128, 0:CH])
                nc.sync.dma_start(
                    out[m * 128 : (m + 1) * 128, c * CH : (c + 1) * CH], ot[:]
                )


def _get_nc():
    if "nc" not in _CACHE:
        _CACHE["nc"] = _build_nc()
    return _CACHE["nc"]


def kernel(x, W_qkv, W_proj, b_proj):
    nc = _get_nc()
    xTs = [
        np.ascontiguousarray(
            x[n].reshape(Q, D).astype(BFNP).T
        )
        for n in range(N)
    ]
    wqs, wks, wvs, wps = [], [], [], []
    for g in range(4):
        c0 = g * GD
        wqs.append(np.ascontiguousarray(W_qkv[:, c0 : c0 + GD].astype(BFNP)))
        wks.append(np.ascontiguousarray(W_qkv[:, D + c0 : D + c0 + GD].astype(BFNP)))
        wvs.append(
            np.ascontiguousarray(W_qkv[:, 2 * D + c0 : 2 * D + c0 + GD].astype(BFNP))
        )
        wps.append(np.ascontiguousarray(W_proj[c0 : c0 + GD, :].astype(BFNP)))
    in_maps = [
        {"xT": xTs[c // 4], "wq": wqs[c % 4], "wk": wks[c % 4],
         "wv": wvs[c % 4], "wp": wps[c % 4]}
        for c in range(N_CORES)
    ]
    res = run_bass_kernel_spmd(nc, in_maps, list(range(N_CORES)), trace=TRACE)
    if TRACE:
        LAST_RESULTS["exec_time_ns"] = res.exec_time_ns
        LAST_RESULTS["mean_exec_time_ns"] = res.mean_exec_time_ns
    out = np.empty((N, T, S, D), np.float32)
    for n in range(N):
        acc = res.results[4 * n]["out"]
        for g in range(1, 4):
            acc = acc + res.results[4 * n + g]["out"]
        out[n] = (acc.T + b_proj).reshape(T, S, D)
    return out
